# revision 1
# baseline (speedup 1.0000x reference)
"""JointAngleLoss Trainium2 kernel (8-core data-parallel).

Input : pose23d_pred [524288, 21, 3] float32
Output: scalar float32 loss (matches reference.reference)

Strategy: pure data-parallel over the batch dim; each of 8 NeuronCores handles
65536 rows. Host pre-permutes the input (dtype preserving) into a per-partition
slot layout J[c][jj][f][k] (jj = joint-within-finger, duplicating the 4 shared
joints: 75 floats per row) so that EVERY device-side vector operand is a flat
contiguous fp16 slice - this keeps the DVE in its 2x_1P packed perf mode.
Device pipeline per group:
  DMA fp32 -> ACT cast->fp16 -> DVE bones/crosses/dots (flat fp16 2x)
  -> ACT relu(-v)^2 with fp32 accum_out  +  PE ones-matmul reduces the
  coplanarity products into PSUM fp32 across groups.
Host sums the per-core partials in float64.
"""

import sys

for _p in ("/opt/trn_rl_repo", "/root/.axon_site/_ro/trn_rl_repo"):
    if _p not in sys.path:
        sys.path.append(_p)

import numpy as np

import concourse.bacc as bacc
import concourse.mybir as mybir
from concourse import tile
from concourse.bass_utils import run_bass_kernel_spmd
from contextlib import ExitStack

N_CORES = 8
P = 128          # SBUF partitions
B_FULL = 524288  # total batch
ROW = 75         # 3 comps * 5 joint-slots * 5 fingers (shared joints duplicated)

F16 = mybir.dt.float16
F32 = mybir.dt.float32


def build_bass(rows_per_core: int, K: int, reps: int = 1, hw_loop: int = 1,
               pool_bones: bool = False):
    """rows_per_core = P * K * G.  K = rows per partition slot per group.

    reps>1 unrolls the compute (timing); hw_loop>1 wraps it in a device-side
    For_i (timing; outputs = last iteration's = one correct pass).
    """
    assert rows_per_core % (P * K) == 0
    G = rows_per_core // (P * K)
    FK = ROW * K          # fp32 elems per partition per group (75*K)
    CJ = 25 * K           # joint elems per component (5jj*5f*K)
    CB = 20 * K           # bone elems per component  (4jj*5f*K)
    S5 = 5 * K            # one [f][k] slab
    NCOP = 3 * S5         # coplane products per partition
    NV = 2 * S5           # v values per partition

    nc = bacc.Bacc("TRN2", target_bir_lowering=False, debug=False)

    x = nc.dram_tensor("x", [G, P, FK], F32, kind="ExternalInput")
    cop_out = nc.dram_tensor("cop_out", [1, NCOP], F32, kind="ExternalOutput")
    mask_out = nc.dram_tensor("mask_out", [P, G * reps], F32, kind="ExternalOutput")

    with tile.TileContext(nc) as tc, ExitStack() as ctx:
        xpool = ctx.enter_context(tc.tile_pool(name="xpool", bufs=2))
        hpool = ctx.enter_context(tc.tile_pool(name="hpool", bufs=1))
        bpool = ctx.enter_context(tc.tile_pool(name="bpool", bufs=2))
        rpool = ctx.enter_context(tc.tile_pool(name="rpool", bufs=2))
        tpool = ctx.enter_context(tc.tile_pool(name="tpool", bufs=1))
        vpool = ctx.enter_context(tc.tile_pool(name="vpool", bufs=2))
        spool = ctx.enter_context(tc.tile_pool(name="spool", bufs=1))
        psum = ctx.enter_context(tc.tile_pool(name="psum", bufs=1, space="PSUM"))

        ones = spool.tile([P, 1], F16)
        nc.gpsimd.memset(ones[:], 1.0)
        acc = spool.tile([P, G * reps], F32)
        psum_cop = psum.tile([1, NCOP], F32)

        n_chunks = (NCOP + 511) // 512

        loop_cm = tc.For_i(0, hw_loop, 1) if hw_loop > 1 else None
        if loop_cm is not None:
            loop_cm.__enter__()

        for rep in range(reps):
            for g in range(G):
                first = rep == 0 and g == 0
                last = rep == reps - 1 and g == G - 1

                # ---- load + cast (all flat, split in half for earlier start)
                xt = xpool.tile([P, FK], F32)
                xh = hpool.tile([P, FK], F16)
                half = FK // 2
                for h in range(2):
                    sl = slice(h * half, (h + 1) * half)
                    nc.sync.dma_start(xt[:, sl], x.ap()[g][:, sl])
                    nc.scalar.copy(xh[:, sl], xt[:, sl])

                # ---- bones: B[c][jj][f][k] = J[c][jj+1][f][k]-J[c][jj][f][k]
                bones = bpool.tile([P, 3 * CB], F16)
                beng = nc.gpsimd if pool_bones else nc.vector
                for c in range(3):
                    beng.tensor_sub(
                        bones[:, c * CB : (c + 1) * CB],
                        xh[:, c * CJ + S5 : c * CJ + CJ],
                        xh[:, c * CJ : c * CJ + CB],
                    )

                def bslab(c, jj):  # bone block, flat [P, 5K]
                    o = c * CB + jj * S5
                    return bones[:, o : o + S5]

                # ---- crosses: R_c[qh][f][k], qh: 0=palm 1=mid 2=tip ---------
                # rot[c] = B_{c1}[jj=qh+1]*B_{c2}[jj=qh] - B_{c2}[jj=qh+1]*B_{c1}[jj=qh]
                rot = []
                for c in range(3):
                    c1, c2 = (c + 1) % 3, (c + 2) % 3
                    m1 = tpool.tile([P, NCOP], F16, tag="m1")
                    m2 = tpool.tile([P, NCOP], F16, tag="m2")
                    rc = rpool.tile([P, NCOP], F16, tag=f"rot{c}")
                    nc.vector.tensor_mul(
                        m1[:], bones[:, c1 * CB + S5 : c1 * CB + CB],
                        bones[:, c2 * CB : c2 * CB + NCOP])
                    nc.vector.tensor_mul(
                        m2[:], bones[:, c2 * CB + S5 : c2 * CB + CB],
                        bones[:, c1 * CB : c1 * CB + NCOP])
                    nc.vector.tensor_sub(rc[:], m1[:], m2[:])
                    rot.append(rc)

                # ---- coplane products: (palm_c + mid_c) * b4_c  (all flat) --
                red = vpool.tile([P, NCOP], F16, tag="red")
                for c in range(3):
                    pc = tpool.tile([P, S5], F16, tag="pc")
                    nc.vector.tensor_add(pc[:], rot[c][:, 0:S5], rot[c][:, S5:2 * S5])
                    nc.vector.tensor_mul(
                        red[:, c * S5 : (c + 1) * S5], pc[:], bslab(c, 3))

                # ---- v1 = tip.mid, v2 = palm.mid ----------------------------
                pprod = []
                for c in range(3):
                    pp = tpool.tile([P, NV], F16, tag=f"pp{c}")
                    nc.vector.tensor_mul(
                        pp[:, 0:S5], rot[c][:, 2 * S5 : 3 * S5], rot[c][:, S5 : 2 * S5])
                    nc.vector.tensor_mul(
                        pp[:, S5:NV], rot[c][:, 0:S5], rot[c][:, S5 : 2 * S5])
                    pprod.append(pp)
                vsum = tpool.tile([P, NV], F16, tag="vsum")
                nc.vector.tensor_add(vsum[:], pprod[0][:], pprod[1][:])
                v = vpool.tile([P, NV], F16, tag="v")
                nc.vector.tensor_add(v[:], vsum[:], pprod[2][:])

                # ---- masked squares on ACT: sum(relu(-v)^2) -> acc ----------
                mrelu = vpool.tile([P, NV], F16, tag="mrelu")
                nc.scalar.activation(mrelu[:], v[:], mybir.ActivationFunctionType.Relu,
                                     scale=-1.0)
                sqj = vpool.tile([P, NV], F16, tag="sqj")
                nc.scalar.activation(sqj[:], mrelu[:],
                                     mybir.ActivationFunctionType.Square,
                                     accum_out=acc[:, rep * G + g : rep * G + g + 1])

                # ---- PE reduction of coplane products over partitions -------
                for i in range(n_chunks):
                    lo = 512 * i
                    hi = min(NCOP, lo + 512)
                    nc.tensor.matmul(psum_cop[:, lo:hi], ones[:], red[:, lo:hi],
                                     start=first, stop=last)

        if loop_cm is not None:
            loop_cm.__exit__(None, None, None)

        # ---- epilogue: PSUM -> SBUF -> DRAM ---------------------------------
        cop_sb = spool.tile([1, NCOP], F32)
        nc.scalar.copy(cop_sb[:], psum_cop[:])
        nc.sync.dma_start(cop_out.ap(), cop_sb[:])
        nc.sync.dma_start(mask_out.ap(), acc[:])

    nc.compile()
    return nc, G


def host_planarize(x: np.ndarray, n_cores: int, K: int) -> np.ndarray:
    """[B,21,3] f32 -> [cores, G, P, 75K] f32: slot layout [c][jj:5][f:5][k]."""
    B = x.shape[0]
    R = B // n_cores
    G = R // (P * K)
    xr = x.reshape(n_cores, G, P, K, 21, 3)
    jidx = (np.arange(5) * 4)[:, None] + np.arange(5)[None, :]  # [f, jj]
    xj = xr[:, :, :, :, jidx, :]                 # [cores,G,P,K,f,jj,3]
    xp = xj.transpose(0, 1, 2, 6, 5, 4, 3)       # [cores,G,P,c,jj,f,K]
    return np.ascontiguousarray(xp).reshape(n_cores, G, P, ROW * K)


_CACHE = {}


def _get_nc(rows_per_core: int, K: int):
    key = (rows_per_core, K)
    if key not in _CACHE:
        _CACHE[key] = build_bass(rows_per_core, K)
    return _CACHE[key]


def kernel(pose23d_pred: np.ndarray) -> np.ndarray:
    x = np.asarray(pose23d_pred, dtype=np.float32)
    assert x.shape == (B_FULL, 21, 3), x.shape
    K = 128
    R = B_FULL // N_CORES
    nc, G = _get_nc(R, K)
    xp = host_planarize(x, N_CORES, K)
    in_maps = [{"x": xp[i]} for i in range(N_CORES)]
    res = run_bass_kernel_spmd(nc, in_maps, list(range(N_CORES)))
    total = 0.0
    for r in res.results:
        total += r["cop_out"].astype(np.float64).sum()
        total += r["mask_out"].astype(np.float64).sum()
    return np.float32(total)



# revision 2
# speedup vs baseline: 5.3613x; 5.3613x over previous
"""JointAngleLoss Trainium2 kernel v7 (8-core data-parallel).

Engine schedule per group (K=128, G=4):
  SP  : 3 DMA chunks of x [P, 63K] fp32 (un-duplicated layout [c][j][k])
  ACT : dedup cast fp32->fp16 into 75-slot layout [c][jj][f][k]
        (+ optionally the bones slab-dup copy)
  DVE : bones (3 contiguous subs), [dup copy], m1/m2 (c-fused 3D or per-c),
        rot, pp (fused), vt, v, relu via dual-op tensor_scalar
  PE  : coplanarity trace matmuls (lhsT=b4 block, rhs=[palm;mid]) and
        optionally the relu^2 sum as mrelu x mrelu trace matmuls
Host sums PSUM diagonals + accum columns in float64.
"""
import sys

for _p in ("/opt/trn_rl_repo", "/root/.axon_site/_ro/trn_rl_repo"):
    if _p not in sys.path:
        sys.path.append(_p)

import numpy as np

import concourse.bacc as bacc
import concourse.mybir as mybir
from concourse import tile
from concourse.ap import AP
from concourse.alu_op_type import AluOpType
from concourse.bass_utils import run_bass_kernel_spmd
from contextlib import ExitStack

N_CORES = 8
P = 128
B_FULL = 524288

F16 = mybir.dt.float16
F32 = mybir.dt.float32
ACTF = mybir.ActivationFunctionType


def _ap(base, off, dims):
    return AP(base.tensor, base.offset + off, [list(base.ap[0])] + dims)


def build_v7(rows_per_core, K=128, reps=1, hw_loop=1, mode="full",
             cross="perc", copy34_eng="scalar", relsq="pe",
             dma_chunks=3, bones_bufs=2, rot_bufs=1, xh_bufs=1):
    assert rows_per_core % (P * K) == 0
    G = rows_per_core // (P * K)
    CJ = 21 * K
    FK = 63 * K
    SK = 25 * K
    S5 = 5 * K
    CB = 20 * K
    MR = 9 * S5
    PPN = 6 * S5
    NV = 2 * S5
    NB = S5 // 128
    NVB = NV // 128
    assert S5 % 128 == 0

    nc = bacc.Bacc("TRN2", target_bir_lowering=False, debug=False)

    x = nc.dram_tensor("x", [G, P, FK], F32, kind="ExternalInput")
    ncols = 384 if relsq == "pe" else 256
    cop_out = nc.dram_tensor("cop_out", [P, ncols], F32, kind="ExternalOutput")
    mask_out = nc.dram_tensor("mask_out", [P, G * reps], F32, kind="ExternalOutput")

    with tile.TileContext(nc) as tc, ExitStack() as ctx:
        xpool = ctx.enter_context(tc.tile_pool(name="xpool", bufs=2))
        hpool = ctx.enter_context(tc.tile_pool(name="hpool", bufs=xh_bufs))
        bpool = ctx.enter_context(tc.tile_pool(name="bpool", bufs=bones_bufs))
        mpool = ctx.enter_context(tc.tile_pool(name="mpool", bufs=1))
        rpool = ctx.enter_context(tc.tile_pool(name="rpool", bufs=rot_bufs))
        vpool = ctx.enter_context(tc.tile_pool(name="vpool", bufs=1))
        spool = ctx.enter_context(tc.tile_pool(name="spool", bufs=1))
        psum = ctx.enter_context(tc.tile_pool(name="psum", bufs=1, space="PSUM"))

        acc = spool.tile([P, G * reps], F32)
        psum_M = psum.tile([P, ncols], F32)
        if relsq != "act":
            nc.gpsimd.memset(acc[:], 0.0)

        xt_static = None
        if mode == "nodma":
            xt_static = spool.tile([P, FK], F32)
            nc.gpsimd.memset(xt_static[:], 0.5)

        loop_cm = tc.For_i(0, hw_loop, 1) if hw_loop > 1 else None
        if loop_cm is not None:
            loop_cm.__enter__()

        for rep in range(reps):
            for g in range(G):
                first = rep == 0 and g == 0
                last = rep == reps - 1 and g == G - 1

                xt = xt_static if mode == "nodma" else xpool.tile([P, FK], F32)
                if mode != "nodma":
                    for ch in range(dma_chunks):
                        lo = FK * ch // dma_chunks
                        hi = FK * (ch + 1) // dma_chunks
                        nc.sync.dma_start(xt[:, lo:hi], x.ap()[g][:, lo:hi])
                if mode == "dma":
                    sink = vpool.tile([P, 2], F32, tag="sink")
                    nc.scalar.activation(sink[:], xt[:, 0:2], ACTF.Copy)
                    continue

                xh = hpool.tile([P, 3 * SK], F16)
                xtb = xt[:]
                for c in range(3):
                    src = _ap(xtb, c * CJ, [[K, 5], [4 * K, 5], [1, K]])
                    dst = xh[:, c * SK:(c + 1) * SK].rearrange(
                        "p (jj f k) -> p jj f k", jj=5, f=5, k=K)
                    nc.scalar.activation(dst, src, ACTF.Copy)

                nslab = 5 if cross == "fused" else 3
                bones = bpool.tile([P, nslab * CB], F16, tag="bones")
                for s in range(3):
                    nc.vector.tensor_sub(
                        bones[:, s * CB:(s + 1) * CB],
                        xh[:, s * SK + S5: s * SK + SK],
                        xh[:, s * SK: s * SK + CB])

                bb = bones[:]
                m1 = mpool.tile([P, MR], F16, tag="m1")
                m2 = mpool.tile([P, MR], F16, tag="m2")
                rot = rpool.tile([P, MR], F16, tag="rot")
                if cross == "fused":
                    if copy34_eng == "scalar":
                        nc.scalar.copy(bones[:, 3 * CB:5 * CB], bones[:, 0:2 * CB])
                    else:
                        nc.vector.tensor_copy(bones[:, 3 * CB:5 * CB],
                                              bones[:, 0:2 * CB])
                    cdims = [[CB, 3], [S5, 3], [1, S5]]
                    nc.vector.tensor_mul(
                        m1[:].rearrange("p (c q e) -> p c q e", c=3, q=3, e=S5),
                        _ap(bb, CB + S5, cdims), _ap(bb, 2 * CB, cdims))
                    nc.vector.tensor_mul(
                        m2[:].rearrange("p (c q e) -> p c q e", c=3, q=3, e=S5),
                        _ap(bb, 2 * CB + S5, cdims), _ap(bb, CB, cdims))
                else:
                    qd = [[S5, 3], [1, S5]]
                    for c in range(3):
                        c1, c2 = (c + 1) % 3, (c + 2) % 3
                        nc.vector.tensor_mul(
                            m1[:, c * 3 * S5:(c + 1) * 3 * S5].rearrange(
                                "p (q e) -> p q e", q=3, e=S5),
                            _ap(bb, c1 * CB + S5, qd), _ap(bb, c2 * CB, qd))
                    for c in range(3):
                        c1, c2 = (c + 1) % 3, (c + 2) % 3
                        nc.vector.tensor_mul(
                            m2[:, c * 3 * S5:(c + 1) * 3 * S5].rearrange(
                                "p (q e) -> p q e", q=3, e=S5),
                            _ap(bb, c2 * CB + S5, qd), _ap(bb, c1 * CB, qd))
                nc.vector.tensor_sub(rot[:], m1[:], m2[:])

                rb = rot[:]
                for c in range(3):
                    for jb in range(NB):
                        lhsT = bones[:, c * CB + 3 * S5 + jb * 128:
                                     c * CB + 3 * S5 + (jb + 1) * 128]
                        rhs = _ap(rb, c * 3 * S5 + jb * 128, [[S5, 2], [1, 128]])
                        nc.tensor.matmul(
                            psum_M[:, 0:256], lhsT, rhs,
                            start=(first and c == 0 and jb == 0),
                            stop=(last and c == 2 and jb == NB - 1))

                pp = vpool.tile([P, PPN], F16, tag="pp", bufs=2)
                nc.vector.tensor_mul(
                    pp[:].rearrange("p (h c e) -> p h c e", h=2, c=3, e=S5),
                    _ap(rb, 2 * S5, [[-2 * S5, 2], [3 * S5, 3], [1, S5]]),
                    _ap(rb, S5, [[0, 2], [3 * S5, 3], [1, S5]]))

                pb = pp[:]
                vt = vpool.tile([P, NV], F16, tag="vt")
                v = vpool.tile([P, NV], F16, tag="v", bufs=2)
                hdims = [[3 * S5, 2], [1, S5]]
                v2d = lambda t: t[:].rearrange("p (h e) -> p h e", h=2, e=S5)
                nc.vector.tensor_add(v2d(vt), _ap(pb, 0, hdims), _ap(pb, S5, hdims))
                nc.vector.tensor_add(v2d(v), v2d(vt), _ap(pb, 2 * S5, hdims))

                col = rep * G + g
                if relsq == "act":
                    mrelu = vpool.tile([P, NV], F16, tag="mrelu")
                    sqj = vpool.tile([P, NV], F16, tag="sqj")
                    nc.scalar.activation(mrelu[:], v[:], ACTF.Relu, scale=-1.0)
                    nc.scalar.activation(sqj[:], mrelu[:], ACTF.Square,
                                         accum_out=acc[:, col:col + 1])
                elif relsq == "dve":
                    mrelu = vpool.tile([P, NV], F16, tag="mrelu")
                    sqj = vpool.tile([P, NV], F16, tag="sqj")
                    nc.vector.tensor_scalar(mrelu[:], v[:], -1.0, 0.0,
                                            AluOpType.mult, AluOpType.max)
                    nc.vector.tensor_tensor_reduce(
                        sqj[:], mrelu[:], mrelu[:], 1.0, 0.0,
                        AluOpType.mult, AluOpType.add,
                        accum_out=acc[:, col:col + 1])
                else:  # pe
                    mrelu = vpool.tile([P, NV], F16, tag="mrelu", bufs=2)
                    nc.vector.tensor_scalar(mrelu[:], v[:], -1.0, 0.0,
                                            AluOpType.mult, AluOpType.max)
                    for vb in range(NVB):
                        blk = mrelu[:, vb * 128:(vb + 1) * 128]
                        nc.tensor.matmul(
                            psum_M[:, 256:384], blk, blk,
                            start=(first and vb == 0),
                            stop=(last and vb == NVB - 1))

        if loop_cm is not None:
            loop_cm.__exit__(None, None, None)

        cop_sb = spool.tile([P, ncols], F32)
        if mode != "dma":
            nc.scalar.copy(cop_sb[:], psum_M[:])
        else:
            nc.gpsimd.memset(cop_sb[:], 0.0)
        nc.sync.dma_start(cop_out.ap(), cop_sb[:])
        nc.sync.dma_start(mask_out.ap(), acc[:])

    nc.compile()
    return nc, G


def host_planarize63(x: np.ndarray, n_cores: int, K: int) -> np.ndarray:
    B = x.shape[0]
    R = B // n_cores
    G = R // (P * K)
    xr = x.reshape(n_cores, G, P, K, 21, 3)
    xp = xr.transpose(0, 1, 2, 5, 4, 3)
    return np.ascontiguousarray(xp).reshape(n_cores, G, P, 63 * K)


_CACHE = {}


def _get_nc(rows_per_core: int, K: int):
    key = (rows_per_core, K)
    if key not in _CACHE:
        _CACHE[key] = build_v7(rows_per_core, K)
    return _CACHE[key]


def kernel(pose23d_pred: np.ndarray) -> np.ndarray:
    x = np.asarray(pose23d_pred, dtype=np.float32)
    assert x.shape == (B_FULL, 21, 3), x.shape
    K = 128
    R = B_FULL // N_CORES
    nc, G = _get_nc(R, K)
    xp = host_planarize63(x, N_CORES, K)
    in_maps = [{"x": xp[i]} for i in range(N_CORES)]
    res = run_bass_kernel_spmd(nc, in_maps, list(range(N_CORES)))
    total = 0.0
    for r in res.results:
        M = r["cop_out"].astype(np.float64)
        total += np.trace(M[:, 0:128]) + np.trace(M[:, 128:256])
        if M.shape[1] >= 384:
            total += np.trace(M[:, 256:384])
        total += r["mask_out"].astype(np.float64).sum()
    return np.float32(total)


# revision 3
# speedup vs baseline: 5.6023x; 1.0449x over previous
"""JointAngleLoss Trainium2 kernel v7 (8-core data-parallel).

Engine schedule per group (K=128, G=4):
  SP  : 3 DMA chunks of x [P, 63K] fp32 (un-duplicated layout [c][j][k])
  ACT : dedup cast fp32->fp16 into 75-slot layout [c][jj][f][k]
        (+ optionally the bones slab-dup copy)
  DVE : bones (3 contiguous subs), [dup copy], m1/m2 (c-fused 3D or per-c),
        rot, pp (fused), vt, v, relu via dual-op tensor_scalar
  PE  : coplanarity trace matmuls (lhsT=b4 block, rhs=[palm;mid]) and
        optionally the relu^2 sum as mrelu x mrelu trace matmuls
Host sums PSUM diagonals + accum columns in float64.
"""
import sys

for _p in ("/opt/trn_rl_repo", "/root/.axon_site/_ro/trn_rl_repo"):
    if _p not in sys.path:
        sys.path.append(_p)

import numpy as np

import concourse.bacc as bacc
import concourse.mybir as mybir
from concourse import tile
from concourse.ap import AP
from concourse.alu_op_type import AluOpType
from concourse.bass_utils import run_bass_kernel_spmd
from contextlib import ExitStack

N_CORES = 8
P = 128
B_FULL = 524288

F16 = mybir.dt.float16
F32 = mybir.dt.float32
ACTF = mybir.ActivationFunctionType


def _ap(base, off, dims):
    return AP(base.tensor, base.offset + off, [list(base.ap[0])] + dims)


def build_v7(rows_per_core, K=128, reps=1, hw_loop=1, mode="full",
             cross="perc", copy34_eng="scalar", relsq="pe",
             dma_chunks=3, bones_bufs=2, rot_bufs=1, xh_bufs=1, bones_fused=False):
    assert rows_per_core % (P * K) == 0
    G = rows_per_core // (P * K)
    CJ = 21 * K
    FK = 63 * K
    SK = 25 * K
    S5 = 5 * K
    CB = 20 * K
    MR = 9 * S5
    PPN = 6 * S5
    NV = 2 * S5
    NB = S5 // 128
    NVB = NV // 128
    assert S5 % 128 == 0

    nc = bacc.Bacc("TRN2", target_bir_lowering=False, debug=False)

    x = nc.dram_tensor("x", [G, P, FK], F32, kind="ExternalInput")
    ncols = 384 if relsq == "pe" else 256
    cop_out = nc.dram_tensor("cop_out", [P, ncols], F32, kind="ExternalOutput")
    mask_out = nc.dram_tensor("mask_out", [P, G * reps], F32, kind="ExternalOutput")

    with tile.TileContext(nc) as tc, ExitStack() as ctx:
        xpool = ctx.enter_context(tc.tile_pool(name="xpool", bufs=2))
        hpool = ctx.enter_context(tc.tile_pool(name="hpool", bufs=xh_bufs))
        bpool = ctx.enter_context(tc.tile_pool(name="bpool", bufs=bones_bufs))
        mpool = ctx.enter_context(tc.tile_pool(name="mpool", bufs=1))
        rpool = ctx.enter_context(tc.tile_pool(name="rpool", bufs=rot_bufs))
        vpool = ctx.enter_context(tc.tile_pool(name="vpool", bufs=1))
        spool = ctx.enter_context(tc.tile_pool(name="spool", bufs=1))
        psum = ctx.enter_context(tc.tile_pool(name="psum", bufs=1, space="PSUM"))

        acc = spool.tile([P, G * reps], F32)
        psum_M = psum.tile([P, ncols], F32)
        if relsq != "act":
            nc.gpsimd.memset(acc[:], 0.0)

        xt_static = None
        if mode == "nodma":
            xt_static = spool.tile([P, FK], F32)
            nc.gpsimd.memset(xt_static[:], 0.5)

        loop_cm = tc.For_i(0, hw_loop, 1) if hw_loop > 1 else None
        if loop_cm is not None:
            loop_cm.__enter__()

        for rep in range(reps):
            for g in range(G):
                first = rep == 0 and g == 0
                last = rep == reps - 1 and g == G - 1

                xt = xt_static if mode == "nodma" else xpool.tile([P, FK], F32)
                if mode != "nodma":
                    for ch in range(dma_chunks):
                        lo = FK * ch // dma_chunks
                        hi = FK * (ch + 1) // dma_chunks
                        nc.sync.dma_start(xt[:, lo:hi], x.ap()[g][:, lo:hi])
                if mode == "dma":
                    sink = vpool.tile([P, 2], F32, tag="sink")
                    nc.scalar.activation(sink[:], xt[:, 0:2], ACTF.Copy)
                    continue

                xh = hpool.tile([P, 3 * SK], F16)
                xtb = xt[:]
                for c in range(3):
                    src = _ap(xtb, c * CJ, [[K, 5], [4 * K, 5], [1, K]])
                    dst = xh[:, c * SK:(c + 1) * SK].rearrange(
                        "p (jj f k) -> p jj f k", jj=5, f=5, k=K)
                    nc.scalar.activation(dst, src, ACTF.Copy)

                nslab = 5 if cross == "fused" else 3
                bones = bpool.tile([P, nslab * CB], F16, tag="bones")
                if bones_fused:
                    sd = [[SK, 3], [1, CB]]
                    nc.vector.tensor_sub(
                        bones[:, 0:3 * CB].rearrange("p (c e) -> p c e", c=3, e=CB),
                        _ap(xh[:], S5, sd), _ap(xh[:], 0, sd))
                else:
                    for s in range(3):
                        nc.vector.tensor_sub(
                            bones[:, s * CB:(s + 1) * CB],
                            xh[:, s * SK + S5: s * SK + SK],
                            xh[:, s * SK: s * SK + CB])

                bb = bones[:]
                m1 = mpool.tile([P, MR], F16, tag="m1")
                m2 = mpool.tile([P, MR], F16, tag="m2")
                rot = rpool.tile([P, MR], F16, tag="rot")
                if cross == "fused":
                    if copy34_eng == "scalar":
                        nc.scalar.copy(bones[:, 3 * CB:5 * CB], bones[:, 0:2 * CB])
                    else:
                        nc.vector.tensor_copy(bones[:, 3 * CB:5 * CB],
                                              bones[:, 0:2 * CB])
                    cdims = [[CB, 3], [S5, 3], [1, S5]]
                    nc.vector.tensor_mul(
                        m1[:].rearrange("p (c q e) -> p c q e", c=3, q=3, e=S5),
                        _ap(bb, CB + S5, cdims), _ap(bb, 2 * CB, cdims))
                    nc.vector.tensor_mul(
                        m2[:].rearrange("p (c q e) -> p c q e", c=3, q=3, e=S5),
                        _ap(bb, 2 * CB + S5, cdims), _ap(bb, CB, cdims))
                else:
                    qd = [[S5, 3], [1, S5]]
                    for c in range(3):
                        c1, c2 = (c + 1) % 3, (c + 2) % 3
                        nc.vector.tensor_mul(
                            m1[:, c * 3 * S5:(c + 1) * 3 * S5].rearrange(
                                "p (q e) -> p q e", q=3, e=S5),
                            _ap(bb, c1 * CB + S5, qd), _ap(bb, c2 * CB, qd))
                    for c in range(3):
                        c1, c2 = (c + 1) % 3, (c + 2) % 3
                        nc.vector.tensor_mul(
                            m2[:, c * 3 * S5:(c + 1) * 3 * S5].rearrange(
                                "p (q e) -> p q e", q=3, e=S5),
                            _ap(bb, c2 * CB + S5, qd), _ap(bb, c1 * CB, qd))
                nc.vector.tensor_sub(rot[:], m1[:], m2[:])

                rb = rot[:]
                for c in range(3):
                    for jb in range(NB):
                        lhsT = bones[:, c * CB + 3 * S5 + jb * 128:
                                     c * CB + 3 * S5 + (jb + 1) * 128]
                        rhs = _ap(rb, c * 3 * S5 + jb * 128, [[S5, 2], [1, 128]])
                        nc.tensor.matmul(
                            psum_M[:, 0:256], lhsT, rhs,
                            start=(first and c == 0 and jb == 0),
                            stop=(last and c == 2 and jb == NB - 1))

                pp = vpool.tile([P, PPN], F16, tag="pp", bufs=2)
                nc.vector.tensor_mul(
                    pp[:].rearrange("p (h c e) -> p h c e", h=2, c=3, e=S5),
                    _ap(rb, 2 * S5, [[-2 * S5, 2], [3 * S5, 3], [1, S5]]),
                    _ap(rb, S5, [[0, 2], [3 * S5, 3], [1, S5]]))

                pb = pp[:]
                vt = vpool.tile([P, NV], F16, tag="vt")
                v = vpool.tile([P, NV], F16, tag="v", bufs=2)
                hdims = [[3 * S5, 2], [1, S5]]
                v2d = lambda t: t[:].rearrange("p (h e) -> p h e", h=2, e=S5)
                nc.vector.tensor_add(v2d(vt), _ap(pb, 0, hdims), _ap(pb, S5, hdims))
                nc.vector.tensor_add(v2d(v), v2d(vt), _ap(pb, 2 * S5, hdims))

                col = rep * G + g
                if relsq == "act":
                    mrelu = vpool.tile([P, NV], F16, tag="mrelu")
                    sqj = vpool.tile([P, NV], F16, tag="sqj")
                    nc.scalar.activation(mrelu[:], v[:], ACTF.Relu, scale=-1.0)
                    nc.scalar.activation(sqj[:], mrelu[:], ACTF.Square,
                                         accum_out=acc[:, col:col + 1])
                elif relsq == "dve":
                    mrelu = vpool.tile([P, NV], F16, tag="mrelu")
                    sqj = vpool.tile([P, NV], F16, tag="sqj")
                    nc.vector.tensor_scalar(mrelu[:], v[:], -1.0, 0.0,
                                            AluOpType.mult, AluOpType.max)
                    nc.vector.tensor_tensor_reduce(
                        sqj[:], mrelu[:], mrelu[:], 1.0, 0.0,
                        AluOpType.mult, AluOpType.add,
                        accum_out=acc[:, col:col + 1])
                else:  # pe
                    mrelu = vpool.tile([P, NV], F16, tag="mrelu", bufs=2)
                    nc.vector.tensor_scalar(mrelu[:], v[:], -1.0, 0.0,
                                            AluOpType.mult, AluOpType.max)
                    for vb in range(NVB):
                        blk = mrelu[:, vb * 128:(vb + 1) * 128]
                        nc.tensor.matmul(
                            psum_M[:, 256:384], blk, blk,
                            start=(first and vb == 0),
                            stop=(last and vb == NVB - 1))

        if loop_cm is not None:
            loop_cm.__exit__(None, None, None)

        cop_sb = spool.tile([P, ncols], F32)
        if mode != "dma":
            nc.scalar.copy(cop_sb[:], psum_M[:])
        else:
            nc.gpsimd.memset(cop_sb[:], 0.0)
        nc.sync.dma_start(cop_out.ap(), cop_sb[:])
        nc.sync.dma_start(mask_out.ap(), acc[:])

    nc.compile()
    return nc, G


def host_planarize63(x: np.ndarray, n_cores: int, K: int) -> np.ndarray:
    B = x.shape[0]
    R = B // n_cores
    G = R // (P * K)
    xr = x.reshape(n_cores, G, P, K, 21, 3)
    xp = xr.transpose(0, 1, 2, 5, 4, 3)
    return np.ascontiguousarray(xp).reshape(n_cores, G, P, 63 * K)


_CACHE = {}


def _get_nc(rows_per_core: int, K: int):
    key = (rows_per_core, K)
    if key not in _CACHE:
        _CACHE[key] = build_v7(rows_per_core, K)
    return _CACHE[key]


def kernel(pose23d_pred: np.ndarray) -> np.ndarray:
    x = np.asarray(pose23d_pred, dtype=np.float32)
    assert x.shape == (B_FULL, 21, 3), x.shape
    K = 128
    R = B_FULL // N_CORES
    nc, G = _get_nc(R, K)
    xp = host_planarize63(x, N_CORES, K)
    in_maps = [{"x": xp[i]} for i in range(N_CORES)]
    res = run_bass_kernel_spmd(nc, in_maps, list(range(N_CORES)))
    total = 0.0
    for r in res.results:
        M = r["cop_out"].astype(np.float64)
        total += np.trace(M[:, 0:128]) + np.trace(M[:, 128:256])
        if M.shape[1] >= 384:
            total += np.trace(M[:, 256:384])
        total += r["mask_out"].astype(np.float64).sum()
    return np.float32(total)
